# revision 18
# baseline (speedup 1.0000x reference)
"""Trainium2 Bass kernel for nn_AttentionModel (sparse_attention).

8-core tensor-parallel distribution with a software-pipelined schedule:
 - k/v layer-1 convs run FIRST; their downstream chain (AllReduce of layer-2
   partials, replicated layer-2 epilogues, sharded layer-3 convs, a small
   AllGather of k3/v3, and the folded pw@v3 projection) hides under the q1 pass.
 - q1 is split into two 16-row blocks. Block A's q2 partials + AllReduce +
   replicated epilogue + q3 rows + AllGather pipeline under block B.
 - layer-2 reductions use single AllReduce ops (cheaper than RS+AG pairs on
   this fabric) with epilogues replicated on every core.
 - the 1x1 output projection is folded into v3 (pv3 = pw @ v3, computed
   mid-kernel), so the tail after softmax is just 8 matmuls + bias + DMA.

dtype strategy: fp16 conv inputs/weights/activations/collectives (half the
HBM + wire bytes of fp32, same 1 cyc/row PE rate), fp32 PSUM accumulation and
fp32 softmax. ~1.8e-3 max rel err vs the fp32 reference.
"""
import os
import sys
import numpy as np

for _p in ('/opt/trn_rl_repo', '/root/problem/work'):
    if _p not in sys.path:
        sys.path.insert(0, _p)

import concourse.bass as bass
import concourse.bacc as bacc
import concourse.tile as tile
import concourse.mybir as mybir
from concourse import bass_utils
from concourse.bass_interp import get_hw_module

F32 = mybir.dt.float32
F16 = mybir.dt.float16
I32 = mybir.dt.int32
AF = mybir.ActivationFunctionType
ALU = mybir.AluOpType
AX = mybir.AxisListType

NCORES = 8
_CACHE = {}

Q1B = [(0, 16), (16, 32)]     # q1 out-row blocks
Q2B = [(0, 15), (15, 32)]     # q2 out rows computable after q1 block k
Q3B = [(0, 14), (14, 32)]     # q3 out rows computable after q2 block k


def _rc(s0, s1):
    r = s0
    while r < s1:
        yield (r, min(r + 8, s1))
        r += 8


def _lrelu(nc, sb, src_ap, bias_ap, bias3_ap, out_ap, name):
    """out = max(src + b, 0.3*src + 0.3b)  (LeakyReLU 0.3)."""
    P = src_ap.shape[0]
    free = int(np.prod(src_ap.shape[1:]))
    s = sb.tile([P, free], F32, name=f"{name}_s", tag="epi_s")
    t = sb.tile([P, free], F32, name=f"{name}_t", tag="epi_t")
    nc.scalar.activation(s[:], src_ap, AF.Identity, bias=bias_ap, scale=1.0)
    nc.scalar.activation(t[:], src_ap, AF.Identity, bias=bias3_ap, scale=0.3)
    nc.vector.tensor_tensor(out_ap, s[:], t[:], op=ALU.max)


def build_program():
    nc = bacc.Bacc("TRN2", target_bir_lowering=False, debug=False,
                   enable_asserts=True, num_devices=NCORES)

    xpad_d = nc.dram_tensor("xpad", [16, 128, 34 * 66], F16, kind="ExternalInput")
    xdec_d = nc.dram_tensor("xdec", [16, 128, 4 * 17 * 32], F16, kind="ExternalInput")
    w1q_d = nc.dram_tensor("w1q", [16, 128, 1152], F16, kind="ExternalInput")
    w1k_d = nc.dram_tensor("w1k", [16, 128, 1152], F16, kind="ExternalInput")
    w1v_d = nc.dram_tensor("w1v", [16, 128, 1152], F16, kind="ExternalInput")
    w2q_d = nc.dram_tensor("w2q", [128, 2304], F16, kind="ExternalInput")
    w2k_d = nc.dram_tensor("w2k", [128, 2304], F16, kind="ExternalInput")
    w2v_d = nc.dram_tensor("w2v", [2, 128, 4608], F16, kind="ExternalInput")
    w3q_d = nc.dram_tensor("w3q", [2, 128, 288], F16, kind="ExternalInput")
    w3k_d = nc.dram_tensor("w3k", [2, 128, 288], F16, kind="ExternalInput")
    w3v_d = nc.dram_tensor("w3v", [8, 128, 1152], F16, kind="ExternalInput")
    wp_d = nc.dram_tensor("wp", [8, 128, 1024], F16, kind="ExternalInput")
    bias_d = nc.dram_tensor("bias", [128, 44], F32, kind="ExternalInput")
    bidx_d = nc.dram_tensor("bidx", [65, 1], I32, kind="ExternalInput")
    out_d = nc.dram_tensor("out_shard", [1024, 256], F32, kind="ExternalOutput")
    ident_d = nc.inline_tensor(np.eye(128, dtype=np.float32), name="ident")

    RG = [list(range(NCORES))]

    with tile.TileContext(nc) as tc:
        with (
            tc.tile_pool(name="dram", bufs=1, space="DRAM") as dram,
            tc.tile_pool(name="wpool", bufs=2) as wpool,
            tc.tile_pool(name="xpool", bufs=2) as xpool,
            tc.tile_pool(name="opool", bufs=1) as opool,
            tc.tile_pool(name="ppool", bufs=1, space="PSUM") as ppool,
            tc.tile_pool(name="misc", bufs=1) as misc,
        ):
            # ---------------- collective DRAM buffers (fp16) ----------------
            arkv_in = dram.tile([1280, 119], F16)    # k2 (256) | v2 (1024) partials
            arkv_out = dram.tile([1280, 119], F16)
            agkv_in = dram.tile([32, 325], F16)      # k3 | v3 local shards
            agkv_out = dram.tile([256, 325], F16, addr_space="Shared")
            arq2_in, arq2_out, ag3_in, ag3_out = [], [], [], []
            for k in range(len(Q1B)):
                npos2 = (Q2B[k][1] - Q2B[k][0]) * 64
                npos3 = (Q3B[k][1] - Q3B[k][0]) * 64
                arq2_in.append(dram.tile([256, npos2], F16, name=f"arq2_in{k}"))
                arq2_out.append(dram.tile([256, npos2], F16, name=f"arq2_out{k}"))
                ag3_in.append(dram.tile([32, npos3], F16, name=f"ag3_in{k}"))
                ag3_out.append(dram.tile([256, npos3], F16, name=f"ag3_out{k}", addr_space="Shared"))
            beta_dram = dram.tile([65, 2048], F16)

            biases = misc.tile([128, 44], F32)
            nc.scalar.dma_start(biases[:], bias_d.ap())
            bcol = lambda j: biases[:, j:j + 1]
            ident = misc.tile([128, 128], F32)
            nc.scalar.dma_start(ident[:], ident_d.ap())
            bidx = misc.tile([65, 1], I32)
            nc.scalar.dma_start(bidx[:], bidx_d.ap())

            # warmup collectives: pay first-use setup during the kv pass
            warm_in = dram.tile([128, 4], F32)
            warm_out = dram.tile([1024, 4], F32, addr_space="Shared")
            nc.scalar.dma_start(warm_in[:], bias_d.ap()[:, 0:4])
            nc.gpsimd.collective_compute("AllGather", ALU.bypass, replica_groups=RG,
                                         ins=[warm_in.opt()], outs=[warm_out.opt()])
            warm_ar_in = dram.tile([8, 4], F32)
            warm_ar_out = dram.tile([8, 4], F32)
            nc.scalar.dma_start(warm_ar_in[:], bias_d.ap()[0:8, 0:4])
            nc.gpsimd.collective_compute("AllReduce", ALU.add, replica_groups=RG,
                                         ins=[warm_ar_in.opt()], outs=[warm_ar_out.opt()])

            # early resident weights (scalar queue)
            w2k_sb = opool.tile([128, 2304], F16, name="w2k_sb")
            nc.scalar.dma_start(w2k_sb[:], w2k_d.ap())
            w2v_sb = opool.tile([128, 2 * 4608], F16, name="w2v_sb")
            for vh in range(2):
                nc.scalar.dma_start(w2v_sb[:, 4608 * vh:4608 * vh + 4608], w2v_d.ap()[vh])
            w1q_sb = opool.tile([128, 16 * 1152], F16, name="w1q_sb")
            w3k_sb = opool.tile([128, 2 * 288], F16, name="w3k_sb")
            w3v_sb = opool.tile([128, 8 * 1152], F16, name="w3v_sb")
            wp_sb = opool.tile([128, 8 * 1024], F16, name="wp_sb")
            w2q_sb = opool.tile([128, 2304], F16, name="w2q_sb")
            w3q_sb = opool.tile([128, 2 * 288], F16, name="w3q_sb")

            # ---------------- persistent activation tiles --------------------
            q1_sb = opool.tile([128, 34 * 66], F16, name="q1_sb")
            q1o = q1_sb.rearrange("c (h w) -> c h w", h=34)
            q2full = opool.tile([128, 2 * 34 * 66], F16, name="q2full")
            q2f4 = q2full.rearrange("p (c h w) -> p c h w", c=2, h=34)
            scores_sb = opool.tile([65, 2048], F32, name="scores_sb")
            pv3sb = opool.tile([128, 520], F32, name="pv3sb")
            pv3T = opool.tile([65, 1024], F16, name="pv3T")
            k3f = opool.tile([128, 2 * 65], F16, name="k3f")
            v3fa = opool.tile([128, 8 * 65], F16, name="v3fa")
            pmax = [misc.tile([65, 1], F32, name=f"pmax{k}") for k in range(len(Q1B))]

            # ================ phase KV: k1/v1 over all 16 ic chunks ==========
            # streams spread across queues: xdec on sync, w1k/w1q on scalar,
            # w1v on gpsimd — one queue can't sustain 1.4MB per chunk.
            k1_ps = ppool.tile([128, 512], F32, name="k1_ps", tag="pk")
            v1_ps = ppool.tile([128, 512], F32, name="v1_ps", tag="pv")
            for ic in range(16):
                xd = xpool.tile([128, 4 * 17 * 32], F16, name="xd", tag="xdec", bufs=3)
                nc.sync.dma_start(xd[:], xdec_d.ap()[ic])
                wk = wpool.tile([128, 1152], F16, name="wk", tag="wB")
                nc.scalar.dma_start(wk[:], w1k_d.ap()[ic])
                wv = wpool.tile([128, 1152], F16, name="wv", tag="wC")
                nc.gpsimd.dma_start(wv[:], w1v_d.ap()[ic])
                first, last = (ic == 0), (ic == 15)
                for wt, ps_t in ((wk, k1_ps), (wv, v1_ps)):
                    for tap in range(9):
                        dy, dx = tap // 3, tap % 3
                        # flat-phase trick: [16, 32] output grid == flat 512 slice
                        # of the [17, 32] phase grid (junk only in ignored row/col)
                        off = (2 * (dy % 2) + (dx % 2)) * 544 + (dy // 2) * 32 + (dx // 2)
                        win2 = xd[:, off: off + 512]
                        nc.tensor.matmul(ps_t[:], wt[:, tap * 128:tap * 128 + 128], win2,
                                         start=(first and tap == 0), stop=(last and tap == 8))

            # epilogues write phase-decimated [4, 9, 17] grids so the stride-2
            # layer-2 convs read contiguous flat windows (see flat-phase trick)
            k1_sb = opool.tile([128, 4 * 9 * 17], F16, name="k1_sb")
            k1d = k1_sb.rearrange("c (f h w) -> c f h w", f=4, h=9)
            k1g = k1_ps.rearrange("c (h w) -> c h w", h=16)   # flat [16, 32] grid
            v1_sb = opool.tile([128, 4 * 9 * 17], F16, name="v1_sb")
            v1d = v1_sb.rearrange("c (f h w) -> c f h w", f=4, h=9)
            v1g = v1_ps.rearrange("c (h w) -> c h w", h=16)
            for g, d_, cb, cb3, nm in ((k1g, k1d, 1, 19, "k1e"), (v1g, v1d, 2, 20, "v1e")):
                for a in range(2):
                    nr = 8 if a == 0 else 7
                    for b in range(2):
                        ncol = 16 if b == 0 else 15
                        _lrelu(nc, misc, g[:, a:15:2, b:31:2], bcol(cb), bcol(cb3),
                               d_[:, 2 * a + b, 0:nr, 0:ncol], f"{nm}{a}{b}")
                # wrap cols: k1 col 31 = col 0 (phase b=1 col 15), col 32 = col 1 (b=0 col 16)
                for a in range(2):
                    nr = 8 if a == 0 else 7
                    nc.vector.tensor_copy(d_[:, 2 * a + 0, 0:nr, 16:17], d_[:, 2 * a + 1, 0:nr, 0:1])
                    nc.vector.tensor_copy(d_[:, 2 * a + 1, 0:nr, 15:16], d_[:, 2 * a + 0, 0:nr, 0:1])

            # ---------------- q1 block machinery -----------------------------
            def q1_block(bi, r0, r1):
                ps = ppool.tile([128, 1024], F32, name=f"q1ps{bi}", tag="pq1", bufs=2)
                nrow = r1 - r0
                for ic in range(16):
                    if bi == 0:
                        nc.sync.dma_start(w1q_sb[:, 1152 * ic:1152 * ic + 1152],
                                          w1q_d.ap()[ic])
                    xp = xpool.tile([128, (nrow + 2) * 66], F16,
                                    name=f"xp{bi}", tag="xq", bufs=2)
                    nc.sync.dma_start(xp[:], xpad_d.ap()[ic][:, r0 * 66:(r1 + 2) * 66])
                    x3 = xp.rearrange("c (h w) -> c h w", h=nrow + 2)
                    first, last = (ic == 0), (ic == 15)
                    for tap in range(9):
                        dy, dx = tap // 3, tap % 3
                        wq_t = w1q_sb[:, ic * 1152 + tap * 128: ic * 1152 + tap * 128 + 128]
                        for (c0, c1) in _rc(0, nrow):
                            win = x3[:, c0 + dy: c1 + dy, dx: dx + 64]
                            nc.tensor.matmul(ps[:, c0 * 64: c1 * 64], wq_t, win,
                                             start=(first and tap == 0),
                                             stop=(last and tap == 8))
                    yield ic
                pv = ps.rearrange("c (h w) -> c h w", h=nrow)
                for (c0, c1) in _rc(0, nrow):
                    _lrelu(nc, misc, pv[:, c0:c1, :], bcol(0), bcol(18),
                           q1o[:, r0 + 1 + c0: r0 + 1 + c1, 1:65], f"q1e{bi}{c0}")
                nc.vector.tensor_copy(q1o[:, r0 + 1:r1 + 1, 0:1], q1o[:, r0 + 1:r1 + 1, 64:65])
                nc.vector.tensor_copy(q1o[:, r0 + 1:r1 + 1, 65:66], q1o[:, r0 + 1:r1 + 1, 1:2])
                if r0 == 0:
                    nc.vector.tensor_copy(q1o[:, 0:1, :], q1o[:, 2:3, :])
                if r1 == 32:
                    nc.vector.tensor_copy(q1o[:, 33:34, :], q1o[:, 31:32, :])

            def q2_block(bi):
                s0, s1 = Q2B[bi]
                for cc in range(2):
                    for (c0, c1) in _rc(s0, s1):
                        n = (c1 - c0) * 64
                        ps = ppool.tile([128, 512], F32, name=f"q2ps{bi}{cc}{c0}",
                                        tag="pq2", bufs=1)
                        for tap in range(9):
                            dy, dx = tap // 3, tap % 3
                            wslc = w2q_sb[:, tap * 256 + 128 * cc: tap * 256 + 128 * cc + 128]
                            win = q1o[:, c0 + dy: c1 + dy, dx: dx + 64]
                            nc.tensor.matmul(ps[:, 0:n], wslc, win,
                                             start=(tap == 0), stop=(tap == 8))
                        qps = misc.tile([128, 512], F16, name=f"qps{bi}{cc}{c0}",
                                        tag="rss", bufs=2)
                        nc.scalar.copy(qps[:, 0:n], ps[:, 0:n])
                        nc.scalar.dma_start(
                            arq2_in[bi][128 * cc:128 * cc + 128,
                                        (c0 - s0) * 64:(c1 - s0) * 64], qps[:, 0:n])
                nc.gpsimd.collective_compute("AllReduce", ALU.add, replica_groups=RG,
                                             ins=[arq2_in[bi].opt()], outs=[arq2_out[bi].opt()])

            def q2_post(bi):
                """Readback the AllReduced q2 partials; replicated epilogue into q2full."""
                s0, s1 = Q2B[bi]
                npos = (s1 - s0) * 64
                q2r = misc.tile([128, 2 * npos], F16, name=f"q2r{bi}", tag="rsl", bufs=2)
                q2rv = q2r.rearrange("p (c f) -> p c f", c=2)
                nc.gpsimd.dma_start(q2rv, arq2_out[bi].rearrange("(c p) f -> p c f", c=2))
                for (c0, c1) in _rc(s0, s1):
                    for c in range(2):
                        src = q2r[:, c * npos:(c + 1) * npos].rearrange(
                            "p (h w) -> p h w", h=s1 - s0)
                        _lrelu(nc, misc, src[:, c0 - s0:c1 - s0, :], bcol(3 + c), bcol(21 + c),
                               q2f4[:, c, 1 + c0:1 + c1, 1:65], f"q2e{bi}{c}{c0}")
                nc.vector.tensor_copy(q2f4[:, :, s0 + 1:s1 + 1, 0:1],
                                      q2f4[:, :, s0 + 1:s1 + 1, 64:65])
                nc.vector.tensor_copy(q2f4[:, :, s0 + 1:s1 + 1, 65:66],
                                      q2f4[:, :, s0 + 1:s1 + 1, 1:2])
                if s0 == 0:
                    nc.vector.tensor_copy(q2f4[:, :, 0:1, :], q2f4[:, :, 2:3, :])
                if s1 == 32:
                    nc.vector.tensor_copy(q2f4[:, :, 33:34, :], q2f4[:, :, 31:32, :])

            def q3_mms(bi):
                u0, u1 = Q3B[bi]
                npos = (u1 - u0) * 64
                q3l = misc.tile([32, npos], F16, name=f"q3l{bi}", tag="q3l", bufs=2)
                for (c0, c1) in _rc(u0, u1):
                    n = (c1 - c0) * 64
                    ps = ppool.tile([32, 512], F32, name=f"q3ps{bi}{c0}", tag="pq3", bufs=1)
                    for jc in range(2):
                        for tap in range(9):
                            dy, dx = tap // 3, tap % 3
                            w = w3q_sb[:, jc * 288 + tap * 32: jc * 288 + tap * 32 + 32]
                            win = q2f4[:, jc, c0 + dy: c1 + dy, dx: dx + 64]
                            nc.tensor.matmul(ps[:, 0:n], w, win,
                                             start=(jc == 0 and tap == 0),
                                             stop=(jc == 1 and tap == 8))
                    _lrelu(nc, misc, ps[:, 0:n], bcol(15)[0:32], bcol(33)[0:32],
                           q3l[:, (c0 - u0) * 64:(c1 - u0) * 64], f"q3e{bi}{c0}")
                nc.scalar.dma_start(ag3_in[bi][:], q3l[:])

            q3fs = {}

            def ag3_issue(bi):
                u0, u1 = Q3B[bi]
                npos = (u1 - u0) * 64
                nc.gpsimd.collective_compute("AllGather", ALU.bypass, replica_groups=RG,
                                             ins=[ag3_in[bi].opt()], outs=[ag3_out[bi].opt()])
                q3f = misc.tile([128, 2 * npos], F16, name=f"q3f{bi}", tag="q3f", bufs=2)
                nc.gpsimd.dma_start(q3f.rearrange("p (c f) -> p c f", c=2),
                                    ag3_out[bi].rearrange("(c p) f -> p c f", c=2))
                q3fs[bi] = q3f

            def scores_mms(bi):
                u0, u1 = Q3B[bi]
                npos = (u1 - u0) * 64
                col0 = u0 * 64
                q3fv = q3fs[bi].rearrange("p (c f) -> p c f", c=2)
                for o0 in range(0, npos, 512):
                    o1 = min(o0 + 512, npos)
                    ps = ppool.tile([65, 512], F32, name=f"sc{bi}{o0}", tag="pk", bufs=1)
                    for jc in range(2):
                        nc.tensor.matmul(ps[:, 0:o1 - o0], k3f[:, 65 * jc:65 * jc + 65],
                                         q3fv[:, jc, o0:o1],
                                         start=(jc == 0), stop=(jc == 1))
                    nc.scalar.copy(scores_sb[:, col0 + o0:col0 + o1], ps[:, 0:o1 - o0])
                nc.vector.reduce_max(pmax[bi][:], scores_sb[:, col0:col0 + npos], axis=AX.X)

            # ================ schedule ======================================
            gA = q1_block(0, 0, 16)
            next(gA)  # ic0 (fills the k1/v1 epilogue latency)

            # --- k2/v2 partials -> single AllReduce -------------------------
            k1f = k1_sb
            v1f = v1_sb
            for cc in range(2):
                kp = ppool.tile([128, 119], F32, name="kp", tag="pk")
                for tap in range(9):
                    dy, dx = tap // 3, tap % 3
                    off = (2 * (dy % 2) + (dx % 2)) * 153 + (dy // 2) * 17 + (dx // 2)
                    nc.tensor.matmul(kp[:], w2k_sb[:, tap * 256 + 128 * cc: tap * 256 + 128 * cc + 128],
                                     k1f[:, off: off + 119], start=(tap == 0), stop=(tap == 8))
                kps = misc.tile([128, 119], F16, name="kps", tag="rss2", bufs=2)
                nc.scalar.copy(kps[:], kp[:])
                nc.scalar.dma_start(arkv_in[128 * cc:128 * cc + 128, :], kps[:])
            for cc in range(8):
                vp = ppool.tile([128, 119], F32, name="vp", tag="pv")
                for tap in range(9):
                    dy, dx = tap // 3, tap % 3
                    off = (2 * (dy % 2) + (dx % 2)) * 153 + (dy // 2) * 17 + (dx // 2)
                    nc.tensor.matmul(vp[:], w2v_sb[:, 4608 * (cc // 4) + tap * 512 + 128 * (cc % 4):
                                               4608 * (cc // 4) + tap * 512 + 128 * (cc % 4) + 128],
                                     v1f[:, off: off + 119], start=(tap == 0), stop=(tap == 8))
                vps = misc.tile([128, 119], F16, name="vps", tag="rss2", bufs=2)
                nc.scalar.copy(vps[:], vp[:])
                nc.scalar.dma_start(arkv_in[256 + 128 * cc:256 + 128 * cc + 128, :], vps[:])
            nc.gpsimd.collective_compute("AllReduce", ALU.add, replica_groups=RG,
                                         ins=[arkv_in.opt()], outs=[arkv_out.opt()])

            # readbacks + replicated k2/v2 epilogues (scalar/vector work only)
            k2r = misc.tile([128, 2 * 119], F16, name="k2r", tag="rsl0")
            nc.gpsimd.dma_start(k2r.rearrange("p (c f) -> p c f", c=2),
                                arkv_out[0:256, :].rearrange("(c p) f -> p c f", c=2))
            v2r = misc.tile([128, 8 * 119], F16, name="v2r", tag="rsl2")
            nc.gpsimd.dma_start(v2r.rearrange("p (c f) -> p c f", c=8),
                                arkv_out[256:1280, :].rearrange("(c p) f -> p c f", c=8))
            k2w = opool.tile([128, 2 * 112], F16, name="k2w")
            k2w4 = k2w.rearrange("p (c h w) -> p c h w", c=2, h=7)
            v2w = opool.tile([128, 8 * 112], F16, name="v2w")
            v2w4 = v2w.rearrange("p (c h w) -> p c h w", c=8, h=7)
            for c in range(2):
                src = k2r[:, c * 119:(c + 1) * 119].rearrange("p (h w) -> p h w", h=7)
                _lrelu(nc, misc, src[:, :, 0:15], bcol(5 + c), bcol(23 + c),
                       k2w4[:, c, :, 0:15], f"k2e{c}")
            nc.vector.tensor_copy(k2w4[:, :, :, 15:16], k2w4[:, :, :, 0:1])
            for c in range(8):
                src = v2r[:, c * 119:(c + 1) * 119].rearrange("p (h w) -> p h w", h=7)
                _lrelu(nc, misc, src[:, :, 0:15], bcol(7 + c), bcol(25 + c),
                       v2w4[:, c, :, 0:15], f"v2e{c}")
            nc.vector.tensor_copy(v2w4[:, :, :, 15:16], v2w4[:, :, :, 0:1])

            for ic in range(1, 8):
                next(gA)

            # --- k3/v3 (sharded out-channels; need the k2/v2 epilogues) ------
            for jc in range(2):
                nc.scalar.dma_start(w3k_sb[:, 288 * jc:288 * jc + 288], w3k_d.ap()[jc])
            for ic8 in range(8):
                nc.scalar.dma_start(w3v_sb[:, 1152 * ic8:1152 * ic8 + 1152], w3v_d.ap()[ic8])
            k3_ps = ppool.tile([32, 70], F32, name="k3_ps", tag="pk")
            v3_ps = ppool.tile([128, 70], F32, name="v3_ps", tag="pv")
            for jc in range(2):
                for tap in range(9):
                    dy, dx = tap // 3, tap % 3
                    wink = k2w4[:, jc, dy: dy + 5, dx: dx + 14]
                    nc.tensor.matmul(k3_ps[:], w3k_sb[:, jc * 288 + tap * 32:jc * 288 + tap * 32 + 32],
                                     wink, start=(jc == 0 and tap == 0), stop=(jc == 1 and tap == 8))
            for ic8 in range(8):
                for tap in range(9):
                    dy, dx = tap // 3, tap % 3
                    winv = v2w4[:, ic8, dy: dy + 5, dx: dx + 14]
                    nc.tensor.matmul(v3_ps[:], w3v_sb[:, ic8 * 1152 + tap * 128:ic8 * 1152 + tap * 128 + 128],
                                     winv, start=(ic8 == 0 and tap == 0), stop=(ic8 == 7 and tap == 8))

            k3g = k3_ps.rearrange("c (h w) -> c h w", h=5)
            k3_sb = opool.tile([32, 65], F16, name="k3_sb")
            _lrelu(nc, misc, k3g[:, :, 0:13], bcol(16)[0:32], bcol(34)[0:32], k3_sb[:], "k3e")
            v3g = v3_ps.rearrange("c (h w) -> c h w", h=5)
            v3_sb = opool.tile([128, 65], F16, name="v3_sb")
            _lrelu(nc, misc, v3g[:, :, 0:13], bcol(17), bcol(35), v3_sb[:], "v3e")
            nc.scalar.dma_start(agkv_in[:, 0:65], k3_sb[:])
            nc.scalar.dma_start(agkv_in[:, 65:325].rearrange("c (a p) -> c a p", a=4), v3_sb[:])
            nc.gpsimd.collective_compute("AllGather", ALU.bypass, replica_groups=RG,
                                         ins=[agkv_in.opt()], outs=[agkv_out.opt()])
            nc.gpsimd.dma_start(k3f.rearrange("p (c f) -> p c f", c=2),
                                agkv_out[:, 0:65].rearrange("(c p) f -> p c f", c=2))
            for i in range(8):
                nc.gpsimd.dma_start(v3fa[:, 65 * i:65 * i + 65],
                                    agkv_out[32 * i:32 * i + 32, 65:325].rearrange(
                                        "r (a p) -> r a p", a=4))
            for i in range(8):
                nc.scalar.dma_start(wp_sb[:, 1024 * i:1024 * i + 1024], wp_d.ap()[i])
            nc.scalar.dma_start(w2q_sb[:], w2q_d.ap())
            for jc in range(2):
                nc.scalar.dma_start(w3q_sb[:, 288 * jc:288 * jc + 288], w3q_d.ap()[jc])

            for ic in range(8, 16):
                next(gA)
            for _ in gA:  # epilogue A
                pass

            # ---------------- q1 block B + chain A ---------------------------
            gB = q1_block(1, 16, 32)
            next(gB)
            next(gB)
            q2_block(0)
            for ic in range(2, 10):
                next(gB)
            q2_post(0)
            for ic in range(10, 14):
                next(gB)
            q3_mms(0)
            ag3_issue(0)
            for ic in range(14, 16):
                next(gB)
            for _ in gB:
                pass

            # ---------------- tail: chain B + softmax + out -------------------
            scores_mms(0)
            q2_block(1)
            # pv3 = pw @ v3 (replicated) + transpose: fills PE idle during ARq2B
            for j in range(8):
                ppv = ppool.tile([128, 65], F32, name=f"ppv{j}", tag="pv", bufs=1)
                for i in range(8):
                    nc.tensor.matmul(ppv[:], wp_sb[:, 1024 * i + 128 * j: 1024 * i + 128 * j + 128],
                                     v3fa[:, 65 * i:65 * i + 65],
                                     start=(i == 0), stop=(i == 7))
                nc.scalar.copy(pv3sb[:, 65 * j:65 * j + 65], ppv[:])
            for j in range(8):
                tps = ppool.tile([65, 128], F32, name=f"tps{j}", tag="pk", bufs=1)
                nc.tensor.transpose(tps[:], pv3sb[:, 65 * j:65 * j + 65], ident[:])
                nc.scalar.copy(pv3T[:, 128 * j:128 * j + 128], tps[:])
            q2_post(1)
            q3_mms(1)
            ag3_issue(1)
            scores_mms(1)

            # softmax over query axis (replicated)
            gmax = misc.tile([65, 1], F32, name="gmax")
            nc.vector.tensor_tensor(gmax[:], pmax[0][:], pmax[1][:], op=ALU.max)
            negmax = misc.tile([65, 1], F32, name="negmax")
            nc.scalar.activation(negmax[:], gmax[:], AF.Identity, scale=-1.0)
            esum0 = misc.tile([65, 1], F32, name="esum0")
            esum1 = misc.tile([65, 1], F32, name="esum1")
            bexp = misc.tile([65, 2048], F16, name="bexp")
            mid = Q3B[1][0] * 64
            nc.scalar.activation(bexp[:, 0:mid], scores_sb[:, 0:mid], AF.Exp,
                                 bias=negmax[:, 0:1], accum_out=esum0[:, 0:1])
            nc.gpsimd.dma_start(beta_dram[:, 0:mid], bexp[:, 0:mid])
            nc.scalar.activation(bexp[:, mid:2048], scores_sb[:, mid:2048], AF.Exp,
                                 bias=negmax[:, 0:1], accum_out=esum1[:, 0:1])
            nc.gpsimd.dma_start(beta_dram[:, mid:2048], bexp[:, mid:2048])
            esum = misc.tile([65, 1], F32, name="esum")
            nc.vector.tensor_tensor(esum[:], esum0[:], esum1[:], op=ALU.add)
            rsum = misc.tile([65, 1], F32, name="rsum")
            nc.vector.reciprocal(rsum[:], esum[:])

            # indirect gather of MY 256 beta columns (row (m, blk) of (520, 256))
            betaB = misc.tile([65, 256], F16, name="betaB")
            nc.gpsimd.indirect_dma_start(
                out=betaB[:], out_offset=None,
                in_=beta_dram.rearrange("m (b p) -> (m b) p", b=8),
                in_offset=bass.IndirectOffsetOnAxis(ap=bidx[:, 0:1], axis=0))
            nc.vector.tensor_scalar_mul(pv3T[:], pv3T[:], rsum[:, 0:1])

            # out = pv3T @ betaB + pb  (position shard: 256 cols)
            for cc in range(8):
                ops = ppool.tile([128, 256], F32, name=f"ops{cc}", tag="pv", bufs=1)
                nc.tensor.matmul(ops[:], pv3T[:, 128 * cc:128 * cc + 128], betaB[:],
                                 start=True, stop=True)
                out_sb = misc.tile([128, 256], F32, name=f"out_sb{cc}", tag="osb", bufs=2)
                nc.vector.tensor_scalar_add(out_sb[:], ops[:], bcol(36 + cc))
                nc.sync.dma_start(out_d.ap()[128 * cc:128 * cc + 128], out_sb[:])

    nc.compile()
    nc.m = get_hw_module(nc.m)
    return nc


def _prep_inputs(x, qw1, qb1, qw2, qb2, qw3, qb3, kw1, kb1, kw2, kb2, kw3, kb3,
                 vw1, vb1, vw2, vb2, vw3, vb3, pw, pb):
    f = np.float32
    h = np.float16
    x = np.ascontiguousarray(np.asarray(x).reshape(2048, 32, 64), dtype=f)
    xp = np.concatenate([x[:, 1:2], x, x[:, 30:31]], axis=1)
    xp = np.concatenate([xp[:, :, -1:], xp, xp[:, :, :1]], axis=2)
    xpad = np.ascontiguousarray(xp.reshape(16, 128, 34 * 66), dtype=h)
    xdec = np.zeros((16, 128, 4, 17, 32), h)
    xr = x.reshape(16, 128, 32, 64)
    for py in range(2):
        for px in range(2):
            xdec[:, :, 2 * py + px, 0:16, 0:32] = xr[:, :, py::2, px::2]
    xdec = np.ascontiguousarray(xdec.reshape(16, 128, 4 * 17 * 32))

    def conv_w(wt, co_lo, co_n, nchunk):
        ws = np.asarray(wt)[co_lo:co_lo + co_n]           # (co_n, Ci, 3, 3)
        ci = ws.shape[1]
        a = ws.reshape(co_n, nchunk, ci // nchunk, 9)     # (co, ck, ci, tap)
        a = a.transpose(1, 2, 3, 0)                       # (ck, ci, tap, co)
        return np.ascontiguousarray(a.reshape(nchunk, ci // nchunk, 9 * co_n), dtype=h)

    def conv_w_ci(wt, ci_lo):
        ws = np.asarray(wt)[:, ci_lo:ci_lo + 128]         # (co, 128, 3, 3)
        co = ws.shape[0]
        a = ws.reshape(co, 128, 9).transpose(1, 2, 0)     # (ci, tap, co)
        return np.ascontiguousarray(a.reshape(128, 9 * co), dtype=h)

    in_maps = []
    for c in range(NCORES):
        m = {"xpad": xpad, "xdec": xdec}
        m["w1q"] = conv_w(qw1, 128 * c, 128, 16)
        m["w1k"] = conv_w(kw1, 128 * c, 128, 16)
        m["w1v"] = conv_w(vw1, 128 * c, 128, 16)
        m["w2q"] = conv_w_ci(qw2, 128 * c)
        m["w2k"] = conv_w_ci(kw2, 128 * c)
        wv2 = np.asarray(vw2)[:, 128 * c:128 * c + 128]        # (1024co, 128ci, 3, 3)
        wv2 = wv2.reshape(2, 512, 128, 9).transpose(0, 2, 3, 1)  # (half, ci, tap, co512)
        m["w2v"] = np.ascontiguousarray(wv2.reshape(2, 128, 4608), dtype=h)
        m["w3q"] = conv_w(qw3, 32 * c, 32, 2)
        m["w3k"] = conv_w(kw3, 32 * c, 32, 2)
        m["w3v"] = conv_w(vw3, 128 * c, 128, 8)
        m["wp"] = np.ascontiguousarray(
            np.asarray(pw)[:, :, 0, 0].T.reshape(8, 128, 1024), dtype=h)
        bias = np.zeros((128, 44), f)
        bias[:, 0] = qb1[128 * c:128 * c + 128]
        bias[:, 1] = kb1[128 * c:128 * c + 128]
        bias[:, 2] = vb1[128 * c:128 * c + 128]
        bias[:, 3] = qb2[0:128]
        bias[:, 4] = qb2[128:256]
        bias[:, 5] = kb2[0:128]
        bias[:, 6] = kb2[128:256]
        for j in range(8):
            bias[:, 7 + j] = vb2[128 * j:128 * j + 128]
        bias[0:32, 15] = qb3[32 * c:32 * c + 32]
        bias[0:32, 16] = kb3[32 * c:32 * c + 32]
        bias[:, 17] = vb3[128 * c:128 * c + 128]
        bias[:, 18:36] = 0.3 * bias[:, 0:18]
        for j in range(8):
            bias[:, 36 + j] = pb[128 * j:128 * j + 128]
        m["bias"] = bias
        m["bidx"] = np.arange(65, dtype=np.int32).reshape(65, 1) * 8 + c
        in_maps.append(m)
    return in_maps


LAST_RESULT = None


def kernel(**inputs):
    global LAST_RESULT
    if "nc" not in _CACHE:
        _CACHE["nc"] = build_program()
    nc = _CACHE["nc"]
    in_maps = _prep_inputs(**{k: np.asarray(v) for k, v in inputs.items()})
    res = bass_utils.run_bass_kernel_spmd(nc, in_maps, core_ids=list(range(NCORES)))
    LAST_RESULT = res
    out = np.empty((1024, 32, 64), np.float32)
    for c in range(NCORES):
        out[:, 4 * c:4 * c + 4, :] = res.results[c]["out_shard"].reshape(1024, 4, 64)
    return np.ascontiguousarray(out.reshape(1, 1024, 32, 64))
